# revision 25
# baseline (speedup 1.0000x reference)
"""Bass/Trainium2 kernel for a single-head causal decoder attention head.

Reference (fp32):
    k = x @ Wk; q = x @ Wq; v = x @ Wv            # [B,T,H]
    att = softmax(causal(q k^T / sqrt(H)))        # [B,T,T]
    out = att @ v                                 # [B,T,H]
with B=4, T=4096, C=1024, H=128.

Sharding: 8 cores = 4 batches x 2 query-interleave lanes (j in {0,1}).
q-blocks are 256 wide (16 blocks); lane j owns blocks {2i+j}.  The host
permutes x^T columns per lane (lane 1 swaps adjacent 256-col blocks) so
that in *position* space every core's slot i has its q-block at position
2i and a causal kv window of positions [0, 2i+2) == kv cols [0, 512(i+1)).
All 8 cores run one uniform SPMD program; lane differences live entirely
in data (the column permutation and a [128, 4*256] mask tile).

Per-core dataflow (transposed land, kv on partitions):
    per step w (one 512-col tg): KT chunk = Wk^T xg, V chunk directly in
    [kv,H] blocks (lhsT = xg cols), Q block = Wq^T xg[:, :256];
    V is stored as fp8e4 (VV8) + bf16 for kv<1024 (VVb).
    attention (slot i, batches of 4 kv-chunks):
      S^T  = KT_c^T Q_i                  (PSUM [128kv, 4, 256q] f32)
      P^T  = exp(S^T/sqrt(H) - 3)        (ACT -> fp8 slots>=2, bf16 else)
      P^T *= mask                        (last batch only, DVE)
      out  += V_c^T P^T ; sums += 1^T P^T
        (fp8 DoubleRow pairs for slots>=2, bf16 for slots 0,1;
         out/sums share one PSUM bank: first PV starts the bank zero,
         everything else accumulates with start=False)
    out/sums -> DRAM via reciprocal + multiply.
Slot 7 is processed incrementally (batch w-1 during step w) so the tail
after the last projection is a single batch.  Projection matmuls are
woven between attention batches to cover ACT exp latency.  All inputs
are host-swizzled to SBUF layout ([128, ...] partition-major) so every
DMA moves large contiguous per-partition segments; the K/Q weights ride
the sync queue ahead of the x stream because the scalar HWDGE queue
starts ~2us later (behind the ACT table load).
"""

import sys

sys.path.insert(0, "/opt/trn_rl_repo")

import numpy as np
import ml_dtypes

import concourse.bass as bass
import concourse.mybir as mybir
import concourse.tile as tile
from concourse import bacc
from concourse.alu_op_type import AluOpType
from concourse.masks import make_identity
from concourse.bass_utils import run_bass_kernel_spmd

B, T, C, H = 4, 4096, 1024, 128
NCORES = 8
QG = 256                      # q-block width
NSLOT = 8                     # slots (q-blocks) per core
CB = C // 128                 # 8 contraction chunks
TGW = 512                     # projection column-group width (2 positions)
NTG = T // TGW                # 8
SCALE = float(H) ** -0.5
EXPBIAS = -3.0

BF16 = mybir.dt.bfloat16
FP8 = mybir.dt.float8e4
F32 = mybir.dt.float32
NPBF16 = ml_dtypes.bfloat16
DR = mybir.MatmulPerfMode.DoubleRow
EXP = mybir.ActivationFunctionType.Exp


def _build_program():
    nc = bacc.Bacc("TRN2", target_bir_lowering=False, debug=False)

    # All inputs are host-swizzled to the exact SBUF layout so every DMA is
    # contiguous per partition (large packets, no gather descriptors).
    xt = nc.dram_tensor("xt", [128, NTG * CB * TGW], BF16, kind="ExternalInput").ap()
    wk = nc.dram_tensor("wk", [128, CB * H], BF16, kind="ExternalInput").ap()
    wq = nc.dram_tensor("wq", [128, CB * H], BF16, kind="ExternalInput").ap()
    wv = nc.dram_tensor("wv", [128, CB * H], BF16, kind="ExternalInput").ap()
    xq7d = nc.dram_tensor("xq7", [128, CB * QG], BF16, kind="ExternalInput").ap()
    msk = nc.dram_tensor("msk", [128, 4 * QG], BF16, kind="ExternalInput").ap()
    outT = nc.dram_tensor("outT", [H, NSLOT * QG], F32, kind="ExternalOutput").ap()

    xtr = xt.rearrange("p (w c t) -> p w c t", w=NTG, c=CB)

    with tile.TileContext(nc) as tc:
        with (
            tc.tile_pool(name="const", bufs=1) as constp,
            tc.tile_pool(name="kvq", bufs=1) as kvqp,
            tc.tile_pool(name="xin", bufs=2) as xinp,
            tc.tile_pool(name="attb", bufs=3) as attp,
            tc.tile_pool(name="epi", bufs=2) as epip,
            tc.tile_pool(name="sp", bufs=2, space="PSUM") as spool,
            tc.tile_pool(name="cp", bufs=2, space="PSUM") as cpool,
            tc.tile_pool(name="op", bufs=1, space="PSUM") as opool,
        ):
            # ---- persistent SBUF ----
            # K/Q weights ride the sync queue ahead of the x stream: the
            # scalar queue starts late (DGE bring-up behind the ACT table
            # load), so anything the first matmuls need must not sit there.
            wks = constp.tile([128, CB * H], BF16, tag="wks")
            nc.sync.dma_start(wks, wk)
            wqs = constp.tile([128, CB * H], BF16, tag="wqs")
            wvs = constp.tile([128, CB * H], BF16, tag="wvs")
            xq7 = constp.tile([128, CB * QG], BF16, tag="xq7")
            nc.scalar.dma_start(xq7, xq7d)
            nc.scalar.dma_start(wvs, wv)

            def wkc(c):
                return wks[:, c * H:(c + 1) * H]
            masks = constp.tile([128, 4 * QG], BF16, tag="masks")
            nc.gpsimd.dma_start(masks, msk)

            onesb = constp.tile([128, H], BF16, tag="onesb")
            nc.vector.memset(onesb, 1.0)
            biast = constp.tile([128, 1], F32, tag="biast")
            nc.vector.memset(biast, EXPBIAS)
            ident = constp.tile([128, 128], BF16, tag="ident")
            make_identity(nc, ident)
            ones8 = constp.tile([128, 2 * H], FP8, tag="ones8")
            nc.vector.memset(ones8, 1.0)
            ones8v = ones8.rearrange("p (k h) -> p k h", k=2)

            KT = kvqp.tile([128, T], BF16, tag="KT")
            QT = kvqp.tile([128, NSLOT * QG], BF16, tag="QT")
            VV8 = kvqp.tile([128, (T // 128) * H], FP8, tag="VV8")
            VV8v = VV8.rearrange("p (k h) -> p k h", k=T // 128)
            VVb = kvqp.tile([128, 8 * H], BF16, tag="VVb")
            VVbv = VVb.rearrange("p (k h) -> p k h", k=8)

            # os7: slot-7 out/sums accumulator, lives for the whole kernel.
            os7 = opool.tile([128, 2 * QG], F32, tag="os7")

            # ---------- emission helpers ----------
            def s_batch(i, b, fillers):
                """Emit S matmuls + exp (+mask) for (slot i, batch b).
                Returns (pt, fp8) for the later PV/sums emission."""
                fp8 = i >= 2
                sps = spool.tile([128, 4 * QG], F32, tag="sps")
                qg = QT[:, i * QG:(i + 1) * QG]
                for kb in range(4):
                    ch = 4 * b + kb
                    nc.tensor.matmul(
                        sps[:, kb * QG:(kb + 1) * QG],
                        lhsT=KT[:, ch * 128:(ch + 1) * 128],
                        rhs=qg,
                        start=(kb % 2 == 0),
                        stop=True,
                        skip_group_check=True,
                    )
                    if fillers and kb % 2 == 1:
                        fillers.pop(0)()
                pt = attp.tile(
                    [128, 4 * QG], FP8 if fp8 else BF16,
                    tag="pt8" if fp8 else "ptb",
                    bufs=3 if fp8 else 2,
                )
                nc.scalar.activation(pt, sps, EXP, bias=biast, scale=SCALE)
                if b == i:
                    nc.vector.tensor_tensor(pt, pt, masks, op=AluOpType.mult)
                return pt

            def pv_batch(i, b, pt, os):
                """Emit PV + sums matmuls for (slot i, batch b) into os."""
                if i >= 2:
                    ptv = pt.rearrange("p (k q) -> p k q", k=4)
                    npairs = 2 * (i + 1)
                    for p in range(2):
                        pair = 2 * b + p
                        c0 = 4 * b + 2 * p
                        nc.tensor.matmul(
                            os[:, 0:QG],
                            lhsT=VV8v[:, c0:c0 + 2, :],
                            rhs=ptv[:, 2 * p:2 * p + 2, :],
                            start=(pair == 0),
                            stop=(pair == npairs - 1),
                            perf_mode=DR,
                            skip_group_check=True,
                        )
                    for p in range(2):
                        pair = 2 * b + p
                        nc.tensor.matmul(
                            os[:, QG:2 * QG],
                            lhsT=ones8v,
                            rhs=ptv[:, 2 * p:2 * p + 2, :],
                            start=False,
                            stop=(pair == npairs - 1),
                            perf_mode=DR,
                            skip_group_check=True,
                        )
                else:
                    nch = 4 * (i + 1)
                    for kb in range(4):
                        ch = 4 * b + kb
                        nc.tensor.matmul(
                            os[:, 0:QG],
                            lhsT=VVbv[:, ch, :],
                            rhs=pt[:, kb * QG:(kb + 1) * QG],
                            start=(ch == 0),
                            stop=(ch == nch - 1),
                            skip_group_check=True,
                        )
                    for kb in range(4):
                        ch = 4 * b + kb
                        nc.tensor.matmul(
                            os[:, QG:2 * QG],
                            lhsT=onesb,
                            rhs=pt[:, kb * QG:(kb + 1) * QG],
                            start=False,
                            stop=(ch == nch - 1),
                            skip_group_check=True,
                        )

            def epilogue(i, os):
                rb = epip.tile([128, QG], F32, tag="rb")
                nc.vector.reciprocal_approx_fast(rb, os[:, QG:2 * QG])
                ot = epip.tile([128, QG], F32, tag="ot")
                nc.vector.tensor_tensor(ot, os[:, 0:QG], rb, op=AluOpType.mult)
                nc.scalar.dma_start(outT[:, i * QG:(i + 1) * QG], ot)

            # ---------- main steps ----------
            osc = None
            for w in range(NTG):
                # stream in this step's x columns
                if w == 0:
                    # separate tiles so the first matmuls depend only on the
                    # first small DMA, not the whole x tile
                    xg0a = xinp.tile([128, 2 * TGW], BF16, tag="xg0a", bufs=1)
                    xg0b = xinp.tile([128, 6 * TGW], BF16, tag="xg0b", bufs=1)
                    nc.sync.dma_start(
                        xg0a.rearrange("p (c q) -> p c q", c=2), xtr[:, 0, 0:2, :]
                    )
                    nc.sync.dma_start(wqs, wq)
                    nc.sync.dma_start(
                        xg0b.rearrange("p (c q) -> p c q", c=6), xtr[:, 0, 2:CB, :]
                    )

                    def xc(c):
                        return xg0a[:, c * TGW:(c + 1) * TGW] if c < 2 else \
                            xg0b[:, (c - 2) * TGW:(c - 1) * TGW]
                else:
                    xg = xinp.tile([128, CB * TGW], BF16, tag="xg")
                    nc.sync.dma_start(
                        xg.rearrange("p (c q) -> p c q", c=CB), xtr[:, w]
                    )

                    def xc(c, xg=xg):
                        return xg[:, c * TGW:(c + 1) * TGW]

                # K projection (always first: attention S needs fresh KT)
                kps = cpool.tile([128, TGW], F32, tag="pps")
                for c in range(CB):
                    nc.tensor.matmul(
                        kps,
                        lhsT=wkc(c),
                        rhs=xc(c),
                        start=(c == 0),
                        stop=(c == CB - 1),
                    )
                nc.vector.tensor_copy(KT[:, w * TGW:(w + 1) * TGW], kps)

                if w == 0:
                    # slot-7 Q block, prefetched so slot 7 can run incrementally
                    q7 = cpool.tile([128, TGW], F32, tag="pps")
                    xq7v = xq7.rearrange("p (c q) -> p c q", c=CB)
                    for c in range(CB):
                        nc.tensor.matmul(
                            q7[:, 0:QG],
                            lhsT=wqs[:, c * H:(c + 1) * H],
                            rhs=xq7v[:, c, :],
                            start=(c == 0),
                            stop=(c == CB - 1),
                        )
                    nc.vector.tensor_copy(QT[:, 7 * QG:8 * QG], q7[:, 0:QG])

                # build the filler list: V projection (+ Q projection) pieces.
                # V is computed as VT [H, 512] (efficient N=512 matmuls) and
                # PE-transposed into [kv, H] blocks; the transposes land in
                # the spare half of the Q PSUM bank (bitcast to bf16).
                fillers = []
                vtp = cpool.tile([128, TGW], F32, tag="pps")

                def mk_vt(c0, w=w, vtp=vtp, xc=xc):
                    def emit():
                        for c in range(c0, c0 + 4):
                            nc.tensor.matmul(
                                vtp,
                                lhsT=wvs[:, c * H:(c + 1) * H],
                                rhs=xc(c),
                                start=(c == 0),
                                stop=(c == CB - 1),
                            )
                    return emit

                fillers.append(mk_vt(0))
                fillers.append(mk_vt(4))

                vtb = epip.tile([128, TGW], BF16, tag="vtb")

                def vt_copy(vtp=vtp, vtb=vtb):
                    nc.vector.tensor_copy(vtb, vtp)
                fillers.append(vt_copy)

                qps = cpool.tile([128, TGW], F32, tag="pps")
                tbuf = qps[:, QG:2 * QG].bitcast(BF16)  # [128, 512] bf16 view

                def mk_q(c0, w=w, qps=qps, xc=xc):
                    def emit():
                        for c in range(c0, c0 + 4):
                            nc.tensor.matmul(
                                qps[:, 0:QG],
                                lhsT=wqs[:, c * H:(c + 1) * H],
                                rhs=xc(c)[:, 0:QG],
                                start=(c == 0),
                                stop=(c == CB - 1),
                                skip_group_check=True,
                            )
                    return emit

                has_q = w < NTG - 1
                if has_q:
                    # Q chain first: its c==0 start zeroes the whole bank,
                    # the transposes then accumulate onto pending-zero.
                    fillers.append(mk_q(0))
                    fillers.append(mk_q(4))

                def mk_t(w=w, vtb=vtb, tbuf=tbuf, first=not has_q):
                    def emit():
                        for kb in range(4):
                            nc.tensor.matmul(
                                tbuf[:, kb * 128:(kb + 1) * 128],
                                lhsT=vtb[:, kb * 128:(kb + 1) * 128],
                                rhs=ident,
                                is_transpose=True,
                                start=(first and kb == 0),
                                stop=True,
                                skip_group_check=True,
                            )
                    return emit
                fillers.append(mk_t())

                def v_copy(w=w, tbuf=tbuf):
                    nc.vector.tensor_copy(
                        VV8v[:, 4 * w:4 * w + 4, :],
                        tbuf.rearrange("p (k h) -> p k h", k=4),
                    )
                    if w < 2:
                        nc.vector.tensor_copy(
                            VVbv[:, 4 * w:4 * w + 4, :],
                            tbuf.rearrange("p (k h) -> p k h", k=4),
                        )
                fillers.append(v_copy)

                if has_q:
                    def q_copy(w=w, qps=qps):
                        nc.vector.tensor_copy(
                            QT[:, w * QG:(w + 1) * QG], qps[:, 0:QG]
                        )
                    fillers.append(q_copy)

                # attention work for this step: slot-7 batch (w-1), then all
                # batches of slot w-1, with fillers woven in.
                pend = []  # (i, b, pt, os) waiting for PV emission
                if w >= 1:
                    pend.append((7, w - 1, s_batch(7, w - 1, fillers), os7))
                if w >= 1:
                    i = w - 1
                    osc = opool.tile([128, 2 * QG], F32, tag="osc")
                    for b in range(i + 1):
                        if fillers:
                            fillers.pop(0)()
                        prev = pend.pop(0) if pend else None
                        pend.append((i, b, s_batch(i, b, fillers), osc))
                        if prev is not None:
                            pv_batch(prev[0], prev[1], prev[2], prev[3])
                while fillers:
                    fillers.pop(0)()
                for (pi, pb, ppt, pos) in pend:
                    pv_batch(pi, pb, ppt, pos)
                if w >= 1:
                    epilogue(w - 1, osc)

            # tail: slot 7's final batch + epilogue
            pt = s_batch(7, 7, [])
            pv_batch(7, 7, pt, os7)
            epilogue(7, os7)

    if not nc.is_finalized():
        nc.finalize()
    return nc


_NC_CACHE = None


def _get_program():
    global _NC_CACHE
    if _NC_CACHE is None:
        _NC_CACHE = _build_program()
    return _NC_CACHE


def _make_mask(j: int) -> np.ndarray:
    """[128, 4, QG] multiplicative mask for the last 4 kv-chunks of a slot.

    Chunks 0-1: the slot's own (diagonal) q-block vs kv positions 0..255:
    keep iff kv_within <= q_within (identical for both lanes).
    Chunks 2-3: the partner block at position 2i+1: lane 0's partner is the
    *future* block (all masked), lane 1's is the *past* block (all kept).
    """
    m = np.zeros((128, 4, QG), np.float32)
    u = np.arange(128)[:, None]
    v = np.arange(QG)[None, :]
    for cc in range(2):
        m[:, cc, :] = (128 * cc + u <= v).astype(np.float32)
    m[:, 2:4, :] = float(j)
    return m.reshape(128, 4 * QG).astype(NPBF16)


def _make_in_maps(x, Wk, Wq, Wv):
    def wswiz(w):  # [C, H] -> [128, CB*H], c-chunk-major per partition
        return np.ascontiguousarray(
            w.reshape(CB, 128, H).transpose(1, 0, 2).reshape(128, CB * H)
        ).astype(NPBF16)

    wk16 = wswiz(Wk)
    wq16 = wswiz(Wq)
    wv16 = wswiz(Wv)
    msks = [_make_mask(j) for j in range(2)]

    in_maps = []
    for b in range(B):
        xtb = np.ascontiguousarray(x[b].T).astype(NPBF16)  # [C, T]
        # lane 1 swaps adjacent 256-col blocks so its q-blocks sit at even
        # positions; lane 0 is the identity permutation.
        xsw = np.ascontiguousarray(
            xtb.reshape(C, NSLOT, 2, QG)[:, :, ::-1, :].reshape(C, T)
        )
        for j in range(2):
            xl = xtb if j == 0 else xsw
            # [C, T] -> [128, NTG, CB, TGW] -> flat, contiguous per partition
            xts = np.ascontiguousarray(
                xl.reshape(CB, 128, NTG, TGW)
                .transpose(1, 2, 0, 3)
                .reshape(128, NTG * CB * TGW)
            )
            xq7 = np.ascontiguousarray(
                xl[:, 7 * TGW:7 * TGW + QG]
                .reshape(CB, 128, QG)
                .transpose(1, 0, 2)
                .reshape(128, CB * QG)
            )
            in_maps.append(
                {
                    "xt": xts,
                    "xq7": xq7,
                    "wk": wk16,
                    "wq": wq16,
                    "wv": wv16,
                    "msk": msks[j],
                }
            )
    return in_maps


def _run(inputs: dict, trace: bool = False, trace_kwargs: dict | None = None):
    x = np.asarray(inputs["x"], np.float32)
    Wk = np.asarray(inputs["Wk"], np.float32)
    Wq = np.asarray(inputs["Wq"], np.float32)
    Wv = np.asarray(inputs["Wv"], np.float32)

    nc = _get_program()
    in_maps = _make_in_maps(x, Wk, Wq, Wv)

    res = run_bass_kernel_spmd(
        nc,
        in_maps,
        core_ids=list(range(NCORES)),
        trace=trace,
        **(trace_kwargs or {}),
    )

    out = np.empty((B, T, H), np.float32)
    for core in range(NCORES):
        b, j = divmod(core, 2)
        oT = np.asarray(res.results[core]["outT"], np.float32)  # [H, 8*QG]
        for i in range(NSLOT):
            g = (2 * i + j) * QG
            out[b, g:g + QG, :] = oT[:, i * QG:(i + 1) * QG].T
    return out, res


def kernel(**inputs) -> np.ndarray:
    out, _ = _run(inputs, trace=False)
    return out


# revision 26
# speedup vs baseline: 1.0183x; 1.0183x over previous
"""Bass/Trainium2 kernel for a single-head causal decoder attention head.

Reference (fp32):
    k = x @ Wk; q = x @ Wq; v = x @ Wv            # [B,T,H]
    att = softmax(causal(q k^T / sqrt(H)))        # [B,T,T]
    out = att @ v                                 # [B,T,H]
with B=4, T=4096, C=1024, H=128.

Sharding: 8 cores = 4 batches x 2 query-interleave lanes (j in {0,1}).
q-blocks are 256 wide (16 blocks); lane j owns blocks {2i+j}.  The host
permutes x^T columns per lane (lane 1 swaps adjacent 256-col blocks) so
that in *position* space every core's slot i has its q-block at position
2i and a causal kv window of positions [0, 2i+2) == kv cols [0, 512(i+1)).
All 8 cores run one uniform SPMD program; lane differences live entirely
in data (the column permutation and a [128, 4*256] mask tile).

Per-core dataflow (transposed land, kv on partitions):
    per step w (one 512-col tg): KT chunk = Wk^T xg, V chunk directly in
    [kv,H] blocks (lhsT = xg cols), Q block = Wq^T xg[:, :256];
    V is stored as fp8e4 (VV8) + bf16 for kv<1024 (VVb).
    attention (slot i, batches of 4 kv-chunks):
      S^T  = KT_c^T Q_i                  (PSUM [128kv, 4, 256q] f32)
      P^T  = exp(S^T/sqrt(H) - 3)        (ACT -> fp8 slots>=2, bf16 else)
      P^T *= mask                        (last batch only, DVE)
      out  += V_c^T P^T ; sums += 1^T P^T
        (fp8 DoubleRow pairs for slots>=2, bf16 for slots 0,1;
         out/sums share one PSUM bank: first PV starts the bank zero,
         everything else accumulates with start=False)
    out/sums -> DRAM via reciprocal + multiply.
Slot 7 is processed incrementally (batch w-1 during step w) so the tail
after the last projection is a single batch.  Projection matmuls are
woven between attention batches to cover ACT exp latency.  All inputs
are host-swizzled to SBUF layout ([128, ...] partition-major) so every
DMA moves large contiguous per-partition segments; the K/Q weights ride
the sync queue ahead of the x stream because the scalar HWDGE queue
starts ~2us later (behind the ACT table load).
"""

import sys

sys.path.insert(0, "/opt/trn_rl_repo")

import numpy as np
import ml_dtypes

import concourse.bass as bass
import concourse.mybir as mybir
import concourse.tile as tile
from concourse import bacc
from concourse.alu_op_type import AluOpType
from concourse.masks import make_identity
from concourse.bass_utils import run_bass_kernel_spmd

B, T, C, H = 4, 4096, 1024, 128
NCORES = 8
QG = 256                      # q-block width
NSLOT = 8                     # slots (q-blocks) per core
CB = C // 128                 # 8 contraction chunks
TGW = 512                     # projection column-group width (2 positions)
NTG = T // TGW                # 8
SCALE = float(H) ** -0.5
EXPBIAS = -3.0

BF16 = mybir.dt.bfloat16
FP8 = mybir.dt.float8e4
F32 = mybir.dt.float32
NPBF16 = ml_dtypes.bfloat16
DR = mybir.MatmulPerfMode.DoubleRow
EXP = mybir.ActivationFunctionType.Exp


def _build_program():
    nc = bacc.Bacc("TRN2", target_bir_lowering=False, debug=False)

    # All inputs are host-swizzled to the exact SBUF layout so every DMA is
    # contiguous per partition (large packets, no gather descriptors).
    xt = nc.dram_tensor("xt", [128, NTG * CB * TGW], BF16, kind="ExternalInput").ap()
    wk = nc.dram_tensor("wk", [128, CB * H], BF16, kind="ExternalInput").ap()
    wq = nc.dram_tensor("wq", [128, CB * H], BF16, kind="ExternalInput").ap()
    wv = nc.dram_tensor("wv", [128, CB * H], BF16, kind="ExternalInput").ap()
    xq7d = nc.dram_tensor("xq7", [128, CB * QG], BF16, kind="ExternalInput").ap()
    msk = nc.dram_tensor("msk", [128, 4 * QG], BF16, kind="ExternalInput").ap()
    outT = nc.dram_tensor("outT", [H, NSLOT * QG], F32, kind="ExternalOutput").ap()

    xtr = xt.rearrange("p (w c t) -> p w c t", w=NTG, c=CB)

    with tile.TileContext(nc) as tc:
        with (
            tc.tile_pool(name="const", bufs=1) as constp,
            tc.tile_pool(name="kvq", bufs=1) as kvqp,
            tc.tile_pool(name="xin", bufs=2) as xinp,
            tc.tile_pool(name="attb", bufs=3) as attp,
            tc.tile_pool(name="epi", bufs=2) as epip,
            tc.tile_pool(name="sp", bufs=2, space="PSUM") as spool,
            tc.tile_pool(name="cp", bufs=2, space="PSUM") as cpool,
            tc.tile_pool(name="op", bufs=1, space="PSUM") as opool,
        ):
            # ---- persistent SBUF ----
            # K/Q weights ride the sync queue ahead of the x stream: the
            # scalar queue starts late (DGE bring-up behind the ACT table
            # load), so anything the first matmuls need must not sit there.
            wks = constp.tile([128, CB * H], BF16, tag="wks")
            nc.sync.dma_start(wks, wk)
            wqs = constp.tile([128, CB * H], BF16, tag="wqs")
            wvs = constp.tile([128, CB * H], BF16, tag="wvs")
            xq7 = constp.tile([128, CB * QG], BF16, tag="xq7")
            nc.scalar.dma_start(xq7, xq7d)
            nc.scalar.dma_start(wvs, wv)

            def wkc(c):
                return wks[:, c * H:(c + 1) * H]
            masks = constp.tile([128, 4 * QG], BF16, tag="masks")
            nc.gpsimd.dma_start(masks, msk)

            onesb = constp.tile([128, H], BF16, tag="onesb")
            nc.vector.memset(onesb, 1.0)
            biast = constp.tile([128, 1], F32, tag="biast")
            nc.vector.memset(biast, EXPBIAS)
            ident = constp.tile([128, 128], BF16, tag="ident")
            make_identity(nc, ident)
            ones8 = constp.tile([128, 2 * H], FP8, tag="ones8")
            nc.vector.memset(ones8, 1.0)
            ones8v = ones8.rearrange("p (k h) -> p k h", k=2)

            KT = kvqp.tile([128, T], BF16, tag="KT")
            QT = kvqp.tile([128, NSLOT * QG], BF16, tag="QT")
            VV8 = kvqp.tile([128, (T // 128) * H], FP8, tag="VV8")
            VV8v = VV8.rearrange("p (k h) -> p k h", k=T // 128)
            VVb = kvqp.tile([128, 8 * H], BF16, tag="VVb")
            VVbv = VVb.rearrange("p (k h) -> p k h", k=8)

            # os7: slot-7 out/sums accumulator, lives for the whole kernel.
            os7 = opool.tile([128, 2 * QG], F32, tag="os7")

            # ---------- emission helpers ----------
            def s_batch(i, b, fillers):
                """Emit S matmuls + exp (+mask) for (slot i, batch b).
                Returns (pt, fp8) for the later PV/sums emission."""
                fp8 = i >= 2
                sps = spool.tile([128, 4 * QG], F32, tag="sps")
                qg = QT[:, i * QG:(i + 1) * QG]
                for kb in range(4):
                    ch = 4 * b + kb
                    nc.tensor.matmul(
                        sps[:, kb * QG:(kb + 1) * QG],
                        lhsT=KT[:, ch * 128:(ch + 1) * 128],
                        rhs=qg,
                        start=(kb % 2 == 0),
                        stop=True,
                        skip_group_check=True,
                    )
                    if fillers and kb % 2 == 1:
                        fillers.pop(0)()
                pt = attp.tile(
                    [128, 4 * QG], FP8 if fp8 else BF16,
                    tag="pt8" if fp8 else "ptb",
                    bufs=3 if fp8 else 2,
                )
                nc.scalar.activation(pt, sps, EXP, bias=biast, scale=SCALE)
                if b == i:
                    nc.vector.tensor_tensor(pt, pt, masks, op=AluOpType.mult)
                return pt

            def pv_batch(i, b, pt, os):
                """Emit PV + sums matmuls for (slot i, batch b) into os."""
                if i >= 2:
                    ptv = pt.rearrange("p (k q) -> p k q", k=4)
                    npairs = 2 * (i + 1)
                    for p in range(2):
                        pair = 2 * b + p
                        c0 = 4 * b + 2 * p
                        nc.tensor.matmul(
                            os[:, 0:QG],
                            lhsT=VV8v[:, c0:c0 + 2, :],
                            rhs=ptv[:, 2 * p:2 * p + 2, :],
                            start=(pair == 0),
                            stop=(pair == npairs - 1),
                            perf_mode=DR,
                            skip_group_check=True,
                        )
                    for p in range(2):
                        pair = 2 * b + p
                        nc.tensor.matmul(
                            os[:, QG:2 * QG],
                            lhsT=ones8v,
                            rhs=ptv[:, 2 * p:2 * p + 2, :],
                            start=False,
                            stop=(pair == npairs - 1),
                            perf_mode=DR,
                            skip_group_check=True,
                        )
                else:
                    nch = 4 * (i + 1)
                    for kb in range(4):
                        ch = 4 * b + kb
                        nc.tensor.matmul(
                            os[:, 0:QG],
                            lhsT=VVbv[:, ch, :],
                            rhs=pt[:, kb * QG:(kb + 1) * QG],
                            start=(ch == 0),
                            stop=(ch == nch - 1),
                            skip_group_check=True,
                        )
                    for kb in range(4):
                        ch = 4 * b + kb
                        nc.tensor.matmul(
                            os[:, QG:2 * QG],
                            lhsT=onesb,
                            rhs=pt[:, kb * QG:(kb + 1) * QG],
                            start=False,
                            stop=(ch == nch - 1),
                            skip_group_check=True,
                        )

            def epilogue(i, os):
                rb = epip.tile([128, QG], F32, tag="rb")
                nc.vector.reciprocal_approx_fast(rb, os[:, QG:2 * QG])
                ot = epip.tile([128, QG], F32, tag="ot")
                nc.vector.tensor_tensor(ot, os[:, 0:QG], rb, op=AluOpType.mult)
                nc.scalar.dma_start(outT[:, i * QG:(i + 1) * QG], ot)

            # ---------- main steps ----------
            osc = None
            for w in range(NTG):
                # stream in this step's x columns
                if w == 0:
                    # separate tiles so the first matmuls depend only on the
                    # first small DMA, not the whole x tile
                    xg0a = xinp.tile([128, 2 * TGW], BF16, tag="xg0a", bufs=1)
                    xg0b = xinp.tile([128, 6 * TGW], BF16, tag="xg0b", bufs=1)
                    nc.sync.dma_start(
                        xg0a.rearrange("p (c q) -> p c q", c=2), xtr[:, 0, 0:2, :]
                    )
                    nc.sync.dma_start(wqs, wq)
                    nc.sync.dma_start(
                        xg0b.rearrange("p (c q) -> p c q", c=6), xtr[:, 0, 2:CB, :]
                    )

                    def xc(c):
                        return xg0a[:, c * TGW:(c + 1) * TGW] if c < 2 else \
                            xg0b[:, (c - 2) * TGW:(c - 1) * TGW]
                else:
                    xg = xinp.tile([128, CB * TGW], BF16, tag="xg")
                    nc.sync.dma_start(
                        xg.rearrange("p (c q) -> p c q", c=CB), xtr[:, w]
                    )

                    def xc(c, xg=xg):
                        return xg[:, c * TGW:(c + 1) * TGW]

                # K projection (always first: attention S needs fresh KT)
                kps = cpool.tile([128, TGW], F32, tag="pps")
                for c in range(CB):
                    nc.tensor.matmul(
                        kps,
                        lhsT=wkc(c),
                        rhs=xc(c),
                        start=(c == 0),
                        stop=(c == CB - 1),
                    )
                nc.vector.tensor_copy(KT[:, w * TGW:(w + 1) * TGW], kps)

                if w == 0:
                    # slot-7 Q block, prefetched so slot 7 can run incrementally
                    q7 = cpool.tile([128, TGW], F32, tag="pps")
                    xq7v = xq7.rearrange("p (c q) -> p c q", c=CB)
                    for c in range(CB):
                        nc.tensor.matmul(
                            q7[:, 0:QG],
                            lhsT=wqs[:, c * H:(c + 1) * H],
                            rhs=xq7v[:, c, :],
                            start=(c == 0),
                            stop=(c == CB - 1),
                        )
                    nc.vector.tensor_copy(QT[:, 7 * QG:8 * QG], q7[:, 0:QG])

                # build the filler list: V projection (+ Q projection) pieces
                fillers = []
                vps = cpool.tile([128, TGW], F32, tag="pps")

                def mk_v(kb, w=w, vps=vps, xc=xc):
                    def emit():
                        for c in range(CB):
                            nc.tensor.matmul(
                                vps[:, kb * H:(kb + 1) * H],
                                lhsT=xc(c)[:, kb * 128:(kb + 1) * 128],
                                rhs=wvs[:, c * H:(c + 1) * H],
                                start=(kb == 0 and c == 0),
                                stop=(c == CB - 1),
                                skip_group_check=True,
                            )
                    return emit

                for kb in range(4):
                    fillers.append(mk_v(kb))

                def v_copy(w=w, vps=vps):
                    nc.vector.tensor_copy(
                        VV8v[:, 4 * w:4 * w + 4, :],
                        vps.rearrange("p (k h) -> p k h", k=4),
                    )
                    if w < 2:
                        nc.vector.tensor_copy(
                            VVbv[:, 4 * w:4 * w + 4, :],
                            vps.rearrange("p (k h) -> p k h", k=4),
                        )
                fillers.append(v_copy)

                if w < NTG - 1:
                    qps = cpool.tile([128, TGW], F32, tag="pps")

                    def mk_q(c0, w=w, qps=qps, xc=xc):
                        def emit():
                            for c in range(c0, c0 + 4):
                                nc.tensor.matmul(
                                    qps[:, 0:QG],
                                    lhsT=wqs[:, c * H:(c + 1) * H],
                                    rhs=xc(c)[:, 0:QG],
                                    start=(c == 0),
                                    stop=(c == CB - 1),
                                )
                        return emit
                    fillers.append(mk_q(0))
                    fillers.append(mk_q(4))

                    def q_copy(w=w, qps=qps):
                        nc.vector.tensor_copy(
                            QT[:, w * QG:(w + 1) * QG], qps[:, 0:QG]
                        )
                    fillers.append(q_copy)

                # attention work for this step: slot-7 batch (w-1), then all
                # batches of slot w-1, with fillers woven in.
                pend = []  # (i, b, pt, os) waiting for PV emission
                if w >= 1:
                    pend.append((7, w - 1, s_batch(7, w - 1, fillers), os7))
                if w >= 1:
                    i = w - 1
                    osc = opool.tile([128, 2 * QG], F32, tag="osc")
                    for b in range(i + 1):
                        if fillers:
                            fillers.pop(0)()
                        prev = pend.pop(0) if pend else None
                        pend.append((i, b, s_batch(i, b, fillers), osc))
                        if prev is not None:
                            pv_batch(prev[0], prev[1], prev[2], prev[3])
                while fillers:
                    fillers.pop(0)()
                for (pi, pb, ppt, pos) in pend:
                    pv_batch(pi, pb, ppt, pos)
                if w >= 1:
                    epilogue(w - 1, osc)

            # tail: slot 7's final batch + epilogue
            pt = s_batch(7, 7, [])
            pv_batch(7, 7, pt, os7)
            epilogue(7, os7)

    if not nc.is_finalized():
        nc.finalize()
    return nc


_NC_CACHE = None


def _get_program():
    global _NC_CACHE
    if _NC_CACHE is None:
        _NC_CACHE = _build_program()
    return _NC_CACHE


def _make_mask(j: int) -> np.ndarray:
    """[128, 4, QG] multiplicative mask for the last 4 kv-chunks of a slot.

    Chunks 0-1: the slot's own (diagonal) q-block vs kv positions 0..255:
    keep iff kv_within <= q_within (identical for both lanes).
    Chunks 2-3: the partner block at position 2i+1: lane 0's partner is the
    *future* block (all masked), lane 1's is the *past* block (all kept).
    """
    m = np.zeros((128, 4, QG), np.float32)
    u = np.arange(128)[:, None]
    v = np.arange(QG)[None, :]
    for cc in range(2):
        m[:, cc, :] = (128 * cc + u <= v).astype(np.float32)
    m[:, 2:4, :] = float(j)
    return m.reshape(128, 4 * QG).astype(NPBF16)


def _make_in_maps(x, Wk, Wq, Wv):
    def wswiz(w):  # [C, H] -> [128, CB*H], c-chunk-major per partition
        return np.ascontiguousarray(
            w.reshape(CB, 128, H).transpose(1, 0, 2).reshape(128, CB * H)
        ).astype(NPBF16)

    wk16 = wswiz(Wk)
    wq16 = wswiz(Wq)
    wv16 = wswiz(Wv)
    msks = [_make_mask(j) for j in range(2)]

    in_maps = []
    for b in range(B):
        xtb = np.ascontiguousarray(x[b].T).astype(NPBF16)  # [C, T]
        # lane 1 swaps adjacent 256-col blocks so its q-blocks sit at even
        # positions; lane 0 is the identity permutation.
        xsw = np.ascontiguousarray(
            xtb.reshape(C, NSLOT, 2, QG)[:, :, ::-1, :].reshape(C, T)
        )
        for j in range(2):
            xl = xtb if j == 0 else xsw
            # [C, T] -> [128, NTG, CB, TGW] -> flat, contiguous per partition
            xts = np.ascontiguousarray(
                xl.reshape(CB, 128, NTG, TGW)
                .transpose(1, 2, 0, 3)
                .reshape(128, NTG * CB * TGW)
            )
            xq7 = np.ascontiguousarray(
                xl[:, 7 * TGW:7 * TGW + QG]
                .reshape(CB, 128, QG)
                .transpose(1, 0, 2)
                .reshape(128, CB * QG)
            )
            in_maps.append(
                {
                    "xt": xts,
                    "xq7": xq7,
                    "wk": wk16,
                    "wq": wq16,
                    "wv": wv16,
                    "msk": msks[j],
                }
            )
    return in_maps


def _run(inputs: dict, trace: bool = False, trace_kwargs: dict | None = None):
    x = np.asarray(inputs["x"], np.float32)
    Wk = np.asarray(inputs["Wk"], np.float32)
    Wq = np.asarray(inputs["Wq"], np.float32)
    Wv = np.asarray(inputs["Wv"], np.float32)

    nc = _get_program()
    in_maps = _make_in_maps(x, Wk, Wq, Wv)

    res = run_bass_kernel_spmd(
        nc,
        in_maps,
        core_ids=list(range(NCORES)),
        trace=trace,
        **(trace_kwargs or {}),
    )

    out = np.empty((B, T, H), np.float32)
    for core in range(NCORES):
        b, j = divmod(core, 2)
        oT = np.asarray(res.results[core]["outT"], np.float32)  # [H, 8*QG]
        for i in range(NSLOT):
            g = (2 * i + j) * QG
            out[b, g:g + QG, :] = oT[:, i * QG:(i + 1) * QG].T
    return out, res


def kernel(**inputs) -> np.ndarray:
    out, _ = _run(inputs, trace=False)
    return out
